# revision 6
# baseline (speedup 1.0000x reference)
"""Lorentz MLA attention kernel for Trainium2, sharded over 8 NeuronCores. v2.

Sharding: tensor-parallel over the 16 attention heads (2 heads per core);
the kv_lora latent projection (wkv_a + RMS norm) is computed on a 1/8
sequence slice per core and AllGathered. The output projection wo is
row-parallel: each core produces a partial (2048, 2047) bf16 output; the
host sums the 8 partials and applies the final Lorentz lift.

v2 changes vs baseline:
- everything bf16 on the matmul paths (scores, AV, wo, projections)
- AV is accumulated transposed (aveT[d, q]) with ONE matmul per k-tile
  (Vp stationary) instead of four, killing most LDWEIGHTS traffic
- V' is produced directly in [s, d] layout by swapping the kvb matmul
  operands (host packs v-columns of both heads contiguously)
- the centroid 1/sqrt(|inner|) is computed on the DVE via the bitcast
  fast-inverse-sqrt seed + 2 Newton steps: NO scalar-engine sqrt in the
  attention phase, so the activation table never leaves the Exp set
  (table thrash was ~70us on the baseline)
- rotary uses a signed sin table ([s; -s]) so it is 4 DVE ops per head
- output is written bf16 (halves the 16.8MB output DMA)
"""

import os
import sys
import types

import numpy as np
import ml_dtypes


def _ensure_axon_hooks():
    """Recreate the missing antenv.axon_hooks module so NTFF tracing works."""
    if "antenv.axon_hooks" in sys.modules:
        return
    try:
        import antenv
        from trn_agent_boot.trn_boot import _ntff_profile_via_ctypes

        hook = _ntff_profile_via_ctypes("/opt/axon/libaxon_pjrt.so")
        mod = types.ModuleType("antenv.axon_hooks")
        mod.get_axon_ntff_profile_hook = lambda: hook
        mod.set_axon_ntff_profile_hook = lambda h: None
        sys.modules["antenv.axon_hooks"] = mod
        antenv.axon_hooks = mod
    except Exception:
        pass


_ensure_axon_hooks()

import concourse.bacc as bacc
import concourse.bass as bass
import concourse.tile as tile
from concourse import mybir
import concourse.bass_utils as bass_utils
from concourse.bass_utils import run_bass_kernel_spmd
from concourse.masks import make_identity, make_upper_triangular

# zero-egress container: make the S3 artifact upload in the profile path a no-op
bass_utils.upload_artifacts = lambda tmpdir: tmpdir

F32 = mybir.dt.float32
BF16 = mybir.dt.bfloat16
I32 = mybir.dt.int32
F32R = mybir.dt.float32r
AF = mybir.ActivationFunctionType
ALU = mybir.AluOpType
AX = mybir.AxisListType

N_CORES = 8
P = 128
S = 2048          # sequence length
DIM = 2048        # model dim
NDC = DIM // P    # 16 contraction chunks over DIM
NQT = S // P      # 16 q/k tiles of 128
HPC = 2           # heads per core
NOPE = 128
RSP = 64          # rotary space dim
VSP = 127         # v space dim
KV_RANK = 512
EPS_RMS = 1e-6
EPS_DEN = 1e-8
QH = NOPE + RSP               # 192 q space rows per head
WQ_COLS = HPC * QH            # 384
WB_COLS = HPC * (NOPE + VSP)  # 510
WO_ROWS = HPC * P             # 256
OUT_COLS = DIM - 1            # 2047
SL = S // N_CORES             # 256
GR = KV_RANK + RSP + 1        # gathered rows: kvn + kpe + t_row
MAGIC = 0x5F3759DF            # fast inverse sqrt seed


def _build_program(exp_scale: float, causal: bool):
    nc = bacc.Bacc("TRN2", target_bir_lowering=False, debug=False,
                   num_devices=N_CORES)

    xT_d = nc.dram_tensor("xT", [DIM, S], BF16, kind="ExternalInput")
    wq_d = nc.dram_tensor("wq", [DIM, WQ_COLS], BF16, kind="ExternalInput")
    wkva_d = nc.dram_tensor("wkva", [DIM, KV_RANK + RSP], BF16, kind="ExternalInput")
    wnormT_d = nc.dram_tensor("wnormT", [P, 4], F32, kind="ExternalInput")
    wkvb_d = nc.dram_tensor("wkvb", [KV_RANK + 1, WB_COLS], BF16, kind="ExternalInput")
    wo_d = nc.dram_tensor("wo", [WO_ROWS, OUT_COLS], BF16, kind="ExternalInput")
    cosT_d = nc.dram_tensor("cosT", [RSP, S], BF16, kind="ExternalInput")
    sinT_d = nc.dram_tensor("sinT", [RSP, S], BF16, kind="ExternalInput")  # [s; -s]
    out_d = nc.dram_tensor("out", [S, OUT_COLS], BF16, kind="ExternalOutput")
    xsl_d = nc.dram_tensor("xsl", [DIM, SL], BF16, kind="ExternalInput")
    cossl_d = nc.dram_tensor("cossl", [RSP, SL], BF16, kind="ExternalInput")
    sinsl_d = nc.dram_tensor("sinsl", [RSP, SL], BF16, kind="ExternalInput")
    signc_d = nc.dram_tensor("signc", [P, 1], F32, kind="ExternalInput")
    gin = nc.dram_tensor("gin", [GR, SL], BF16)
    gout = nc.dram_tensor("gout", [N_CORES, GR, SL], BF16, addr_space="Shared")

    with tile.TileContext(nc) as tc:
        const = tc.alloc_tile_pool(name="const", bufs=1)
        identity = const.tile([P, P], F32)
        make_identity(nc, identity)
        diagmask = const.tile([P, P], F32)
        make_upper_triangular(nc, diagmask, val=1.0, diag=True)
        diagmask_bf = const.tile([P, P], BF16)
        nc.vector.tensor_copy(diagmask_bf[:], diagmask[:])
        wnormT = const.tile([P, 4], F32)
        nc.sync.dma_start(out=wnormT[:], in_=wnormT_d[:])
        Lt = const.tile([P, 4, 2], F32)  # [ones | wnorm^2] per latent chunk
        for c in range(4):
            nc.vector.memset(Lt[:, c, 0:1], 1.0)
            nc.vector.tensor_mul(Lt[:, c, 1:2], wnormT[:, c:c + 1], wnormT[:, c:c + 1])
        ones_col = const.tile([P, 1], F32)
        nc.vector.memset(ones_col[:], 1.0)
        ones_col_bf = const.tile([P, 1], BF16)
        nc.vector.memset(ones_col_bf[:], 1.0)
        ones_row_bf = const.tile([1, P], BF16)
        nc.vector.memset(ones_row_bf[:], 1.0)
        ones_row = const.tile([1, P], F32)
        nc.vector.memset(ones_row[:], 1.0)
        eps_b = const.tile([P, 1], F32)
        nc.vector.memset(eps_b[:], EPS_RMS)
        sign_col_bf = const.tile([P, 1], F32)  # +1 except -1 on the time row
        nc.sync.dma_start(out=sign_col_bf[:], in_=signc_d[:])
        magic_i = const.tile([P, 4], I32)
        nc.vector.memset(magic_i[:], MAGIC)

        # Long-lived tensors.
        big = tc.alloc_tile_pool(name="big", bufs=1)
        qsA = [big.tile([P, S], BF16, name=f"qsA_{h}", tag=f"qsA_{h}") for h in range(HPC)]
        qsB = [big.tile([RSP + 1, S], F32R, name=f"qsB_{h}", tag=f"qsB_{h}") for h in range(HPC)]
        kv = [big.tile([P, S], BF16, name=f"kv_{c}", tag=f"kv_{c}") for c in range(4)]
        kpe = big.tile([RSP, S], BF16, name="kpe", tag="kpe")
        t_row_bf = big.tile([1, S], BF16, name="t_row_bf", tag="t_row_bf")

        # ------------- kv latent on this core's s-slice, then AllGather ------
        p_wA = tc.alloc_tile_pool(name="p_wA", bufs=1)
        p_sl = tc.alloc_tile_pool(name="p_sl", bufs=1)
        p_pssl = tc.alloc_tile_pool(name="p_pssl", bufs=2, space="PSUM")
        xsl_t = p_sl.tile([P, NDC, SL], BF16, name="xsl_t", tag="xsl_t")
        wKV = []
        for dc in range(NDC):
            w = p_wA.tile([P, KV_RANK + RSP], BF16, name=f"wKV_{dc}", tag=f"wKV_{dc}")
            nc.sync.dma_start(out=w[:], in_=wkva_d[dc * P:(dc + 1) * P, :])
            nc.sync.dma_start(out=xsl_t[:, dc, :],
                              in_=xsl_d[dc * P:(dc + 1) * P, :])
            wKV.append(w)
        cossl = p_sl.tile([RSP, SL], BF16, name="cossl", tag="cossl")
        sinsl = p_sl.tile([RSP, SL], BF16, name="sinsl", tag="sinsl")
        nc.sync.dma_start(out=cossl[:], in_=cossl_d[:])
        nc.sync.dma_start(out=sinsl[:], in_=sinsl_d[:])

        kvsl = [p_sl.tile([P, SL], F32, name=f"kvsl_{c}", tag=f"kvsl_{c}")
                for c in range(4)]
        kpesl = p_sl.tile([RSP, SL], BF16, name="kpesl", tag="kpesl")
        for c in range(4):
            ps = p_pssl.tile([P, SL], F32, name="psl", tag="psl", bufs=2)
            for dc in range(NDC):
                nc.tensor.matmul(ps[:], wKV[dc][:, c * P:(c + 1) * P],
                                 xsl_t[:, dc, :], start=(dc == 0), stop=(dc == NDC - 1))
            nc.vector.tensor_copy(kvsl[c][:], ps[:])
        ps = p_pssl.tile([P, SL], F32, name="psl", tag="psl", bufs=2)
        for dc in range(NDC):
            nc.tensor.matmul(ps[:RSP, :], wKV[dc][:, KV_RANK:],
                             xsl_t[:, dc, :], start=(dc == 0), stop=(dc == NDC - 1))
        nc.vector.tensor_copy(kpesl[:], ps[:RSP, :])

        # RMS stats on the slice
        ps_s = p_pssl.tile([1, SL], F32, name="ps_s", tag="ps_s", bufs=1)
        ps_w = p_pssl.tile([1, SL], F32, name="ps_w", tag="ps_w", bufs=1)
        for c in range(4):
            ksq = p_sl.tile([P, SL], F32, name="ksq", tag="ksq", bufs=2)
            nc.scalar.square(ksq[:], kvsl[c][:])
            nc.tensor.matmul(ps_s[:], Lt[:, c, 0:1], ksq[:], start=(c == 0), stop=(c == 3))
            nc.tensor.matmul(ps_w[:], Lt[:, c, 1:2], ksq[:], start=(c == 0), stop=(c == 3))
        sq_s = p_sl.tile([1, SL], F32, name="sq_s", tag="sq_s")
        nc.scalar.activation(sq_s[:], ps_s[:], AF.Sqrt, bias=eps_b[0:1, :],
                             scale=1.0 / KV_RANK)
        inv_rms = p_sl.tile([1, SL], F32, name="inv_rms", tag="inv_rms")
        nc.vector.reciprocal(inv_rms[:], sq_s[:])
        tmp_r = p_sl.tile([1, SL], F32, name="tmp_r", tag="tmp_r")
        nc.vector.tensor_copy(tmp_r[:], ps_w[:])
        nc.vector.tensor_mul(tmp_r[:], tmp_r[:], inv_rms[:])
        nc.vector.tensor_mul(tmp_r[:], tmp_r[:], inv_rms[:])
        t_st = p_sl.tile([1, SL], BF16, name="t_st", tag="t_st")
        nc.scalar.activation(t_st[:], tmp_r[:], AF.Sqrt, bias=1.0)

        # broadcast inv_rms via outer product; fused scale -> bf16 stage
        rb = p_pssl.tile([P, SL], F32, name="rb", tag="rb", bufs=1)
        nc.tensor.matmul(rb[:], ones_row[:], inv_rms[:], start=True, stop=True)
        kvn_st = [p_sl.tile([P, SL], BF16, name=f"kvn_st_{c}", tag=f"kvn_st_{c}")
                  for c in range(4)]
        for c in range(4):
            nc.vector.scalar_tensor_tensor(
                kvn_st[c][:], kvsl[c][:], wnormT[:, c:c + 1], rb[:],
                op0=ALU.mult, op1=ALU.mult)

        # rotary on the k_pe slice (4 ops; sinsl rows 32:64 hold -sin)
        rtl = p_sl.tile([RSP, SL], BF16, name="rtl", tag="rtl")
        mrl = p_sl.tile([RSP, SL], BF16, name="mrl", tag="mrl")
        kpe_st = p_sl.tile([RSP, SL], BF16, name="kpe_st", tag="kpe_st")
        nc.vector.tensor_mul(rtl[32:64, :], kpesl[0:32, :], sinsl[0:32, :])
        nc.vector.tensor_mul(rtl[0:32, :], kpesl[32:64, :], sinsl[32:64, :])
        nc.vector.tensor_mul(mrl[:], kpesl[:], cossl[:])
        nc.vector.tensor_add(kpe_st[:], mrl[:], rtl[:])

        # ship slice, gather full (single bf16 payload)
        for c in range(4):
            nc.sync.dma_start(out=gin[c * P:(c + 1) * P, :], in_=kvn_st[c][:])
        nc.sync.dma_start(out=gin[KV_RANK:KV_RANK + RSP, :], in_=kpe_st[:])
        nc.sync.dma_start(out=gin[KV_RANK + RSP:, :], in_=t_st[:])
        nc.gpsimd.collective_compute(
            "AllGather", ALU.bypass,
            replica_groups=[list(range(N_CORES))],
            ins=[gin[:]], outs=[gout[:]])
        H8 = N_CORES // 2
        for c in range(4):
            for half in range(2):
                k0 = half * H8
                nc.sync.dma_start(
                    out=kv[c][:, k0 * SL:(k0 + H8) * SL].rearrange(
                        "p (k s) -> p k s", k=H8),
                    in_=gout[k0:k0 + H8, c * P:(c + 1) * P, :].rearrange(
                        "k p s -> p k s"))
        nc.sync.dma_start(
            out=kpe[:].rearrange("p (k s) -> p k s", k=N_CORES),
            in_=gout[:, KV_RANK:KV_RANK + RSP, :].rearrange("k p s -> p k s"))
        nc.sync.dma_start(
            out=t_row_bf[:].rearrange("p (k s) -> p k s", k=N_CORES),
            in_=gout[:, KV_RANK + RSP:, :].rearrange("k p s -> p k s"))
        p_pssl.release()
        p_sl.release()

        # --- q projection over the full sequence (overlaps the AllGather) ----
        p_qw = tc.alloc_tile_pool(name="p_qw", bufs=1)
        wQ = []
        for dc in range(NDC):
            w = p_qw.tile([P, WQ_COLS], BF16, name=f"wQ_{dc}", tag=f"wQ_{dc}")
            nc.scalar.dma_start(out=w[:], in_=wq_d[dc * P:(dc + 1) * P, :])
            wQ.append(w)
        p_xs = tc.alloc_tile_pool(name="p_xs", bufs=1)
        p_psA = tc.alloc_tile_pool(name="p_psA", bufs=3, space="PSUM")
        xpe = p_qw.tile([P, S], BF16, name="xpe", tag="xpe")  # packed rope pre-rotary

        NCH = 512
        NA = S // NCH
        for n in range(NA):
            xt = p_xs.tile([P, NDC, NCH], BF16, name="xt", tag="xt", bufs=2)
            src = xT_d[:, n * NCH:(n + 1) * NCH].rearrange("(dc p) s -> p dc s", p=P)
            for dc in range(NDC):
                eng = nc.sync if dc % 2 == 0 else nc.scalar
                eng.dma_start(out=xt[:, dc, :], in_=src[:, dc, :])
            # rope chunk first so rotary can start early
            chunks = [(2 * P, xpe), (0, qsA[0]), (P, qsA[1])]
            for (col0, dst) in chunks:
                ps = p_psA.tile([P, NCH], F32, name="psa", tag="psa", bufs=3)
                for dc in range(NDC):
                    nc.tensor.matmul(ps[:], wQ[dc][:, col0:col0 + P],
                                     xt[:, dc, :],
                                     start=(dc == 0), stop=(dc == NDC - 1))
                nc.scalar.copy(dst[:, n * NCH:(n + 1) * NCH], ps[:])
        p_psA.release()
        p_xs.release()

        # ---------------- rotary on q (packed both heads) --------------------
        p_a2 = tc.alloc_tile_pool(name="p_a2", bufs=1)
        cos128 = p_a2.tile([P, S], BF16, name="cos128", tag="cos128")
        sin128 = p_a2.tile([P, S], BF16, name="sin128", tag="sin128")
        for hh in range(2):
            nc.scalar.dma_start(out=cos128[hh * RSP:(hh + 1) * RSP, :], in_=cosT_d[:])
            nc.scalar.dma_start(out=sin128[hh * RSP:(hh + 1) * RSP, :], in_=sinT_d[:])
        mrot = p_a2.tile([P, S], BF16, name="mrot", tag="mrot")
        rt = p_a2.tile([P, S], BF16, name="rt", tag="rt")
        nc.vector.tensor_mul(mrot[:], xpe[:], cos128[:])
        nc.vector.tensor_mul(rt[32:64, :], xpe[0:32, :], sin128[0:32, :])
        nc.vector.tensor_mul(rt[0:32, :], xpe[32:64, :], sin128[32:64, :])
        nc.vector.tensor_mul(rt[96:128, :], xpe[64:96, :], sin128[64:96, :])
        nc.vector.tensor_mul(rt[64:96, :], xpe[96:128, :], sin128[96:128, :])
        nc.vector.tensor_add(qsB[0][0:RSP, :], mrot[0:RSP, :], rt[0:RSP, :])
        nc.vector.tensor_add(qsB[1][0:RSP, :], mrot[RSP:P, :], rt[RSP:P, :])
        p_a2.release()
        p_qw.release()

        # q time rows (independent of the AllGather; fills the wait window)
        p_qt = tc.alloc_tile_pool(name="p_qt", bufs=1)
        p_pkq = tc.alloc_tile_pool(name="p_pkq", bufs=2, space="PSUM")
        qtmp = [p_qt.tile([1, S], F32, name=f"qtmp_{h}", tag=f"qtmp_{h}")
                for h in range(HPC)]
        for h in range(HPC):
            qbigsq = p_qt.tile([P, S], BF16, name="qbigsq", tag="qbigsq", bufs=2)
            nc.vector.tensor_mul(qbigsq[:], qsA[h][:], qsA[h][:])
            smsq = p_qt.tile([RSP, S], BF16, name="smsq", tag="smsq", bufs=2)
            nc.vector.tensor_mul(smsq[:], qsB[h][0:RSP, :], qsB[h][0:RSP, :])
            for n in range(4):
                n0 = n * 512
                pq = p_pkq.tile([1, 512], F32, name="pq", tag="pq", bufs=2)
                nc.tensor.matmul(pq[:], ones_col_bf[:], qbigsq[:, n0:n0 + 512],
                                 start=True, stop=False)
                nc.tensor.matmul(pq[:], ones_col_bf[0:RSP, :], smsq[:, n0:n0 + 512],
                                 start=False, stop=True)
                nc.scalar.activation(qtmp[h][:, n0:n0 + 512], pq[:], AF.Sqrt, bias=1.0)
            nc.vector.tensor_scalar_mul(qsB[h][RSP:RSP + 1, :], qtmp[h][:], -1.0)
        p_pkq.release()
        p_qt.release()

        # ---------------- kv_b projection ------------------------------------
        big2 = tc.alloc_tile_pool(name="big2", bufs=1)
        p_wB = tc.alloc_tile_pool(name="p_wB", bufs=1)
        p_psB = tc.alloc_tile_pool(name="p_psB", bufs=3, space="PSUM")
        p_sc2 = tc.alloc_tile_pool(name="p_sc2", bufs=2)
        wb_k = []
        for k in range(4):
            w = p_wB.tile([P, WB_COLS], BF16, name=f"wbk_{k}", tag=f"wbk_{k}")
            nc.scalar.dma_start(out=w[:], in_=wkvb_d[k * P:(k + 1) * P, :])
            wb_k.append(w)
        wb_t = p_wB.tile([1, WB_COLS], BF16, name="wb_t", tag="wb_t")
        nc.scalar.dma_start(out=wb_t[:], in_=wkvb_d[KV_RANK:KV_RANK + 1, :])

        ksA = [big2.tile([P, S], BF16, name=f"ksA_{h}", tag=f"ksA_{h}") for h in range(HPC)]
        ksB = [big2.tile([RSP + 1, S], F32R, name=f"ksB_{h}", tag=f"ksB_{h}")
               for h in range(HPC)]
        Vp = [big2.tile([P, NQT, P], F32R, name=f"Vp_{h}", tag=f"Vp_{h}")
              for h in range(HPC)]
        vsums = [big2.tile([P, NQT, 1], F32, name=f"vsums_{h}", tag=f"vsums_{h}")
                 for h in range(HPC)]

        # V' blocks, already [s, d]: out = kv_pt_chunk^T @ wb_v
        VCOL = 2 * VSP  # 254, v cols of both heads packed
        for j in range(NQT):
            vps = p_psB.tile([P, VCOL], F32, name="vps", tag="vps", bufs=2)
            for k in range(4):
                nc.tensor.matmul(vps[:], kv[k][:, j * P:(j + 1) * P],
                                 wb_k[k][:, 2 * NOPE:], start=(k == 0), stop=False)
            nc.tensor.matmul(vps[:], t_row_bf[0:1, j * P:(j + 1) * P],
                             wb_t[:, 2 * NOPE:], start=False, stop=True)
            for h in range(HPC):
                if h == 0:
                    nc.scalar.copy(Vp[h][:, j, 0:VSP], vps[:, h * VSP:(h + 1) * VSP])
                else:
                    nc.vector.tensor_copy(Vp[h][:, j, 0:VSP],
                                          vps[:, h * VSP:(h + 1) * VSP])
                vscr = p_sc2.tile([P, VSP], BF16, name="vscr", tag="vscr", bufs=2)
                nc.scalar.activation(vscr[:], vps[:, h * VSP:(h + 1) * VSP],
                                     AF.Square, accum_out=vsums[h][:, j, :])
        for h in range(HPC):
            nc.scalar.activation(Vp[h][:, :, VSP:VSP + 1], vsums[h][:],
                                 AF.Sqrt, bias=1.0)

        # k_nope: out = wb_nope^T @ kv_pt  (stays [d, s] as scores lhsT)
        kpesq = p_sc2.tile([RSP, S], BF16, name="kpesq", tag="kpesq", bufs=1)
        nc.vector.tensor_mul(kpesq[:], kpe[:], kpe[:])
        for h in range(HPC):
            nc.vector.tensor_copy(ksB[h][0:RSP, :], kpe[:])
        p_pk = tc.alloc_tile_pool(name="p_pk", bufs=2, space="PSUM")
        for n in range(4):
            n0 = n * 512
            for h in range(HPC):
                ps = p_psB.tile([P, 512], F32, name="psb", tag="psb", bufs=2)
                for k in range(4):
                    nc.tensor.matmul(ps[:], wb_k[k][:, h * NOPE:(h + 1) * NOPE],
                                     kv[k][:, n0:n0 + 512], start=(k == 0), stop=False)
                nc.tensor.matmul(ps[:], wb_t[:, h * NOPE:(h + 1) * NOPE],
                                 t_row_bf[:, n0:n0 + 512], start=False, stop=True)
                nc.scalar.copy(ksA[h][:, n0:n0 + 512], ps[:])
                sqk = p_sc2.tile([P, 512], BF16, name="sqk", tag="sqk", bufs=2)
                nc.vector.tensor_mul(sqk[:], ksA[h][:, n0:n0 + 512],
                                     ksA[h][:, n0:n0 + 512])
                pk = p_pk.tile([1, 512], F32, name="pk", tag="pk", bufs=2)
                nc.tensor.matmul(pk[:], ones_col_bf[:], sqk[:],
                                 start=True, stop=False)
                nc.tensor.matmul(pk[:], ones_col_bf[0:RSP, :], kpesq[:, n0:n0 + 512],
                                 start=False, stop=True)
                nc.scalar.activation(ksB[h][RSP:RSP + 1, n0:n0 + 512], pk[:],
                                     AF.Sqrt, bias=1.0)

        p_pk.release()
        p_psB.release()
        p_sc2.release()
        p_wB.release()

        # ---------------- attention ------------------------------------------
        # scoresT layout [k, q]; exp -> ex bf16; AV accumulates aveT[d, q] with
        # one matmul per k-tile (Vp stationary). Centroid scale 1/sqrt(|inner|)
        # via DVE bitcast + Newton; cenT = aveT * rsb is already the wo lhsT.
        GQ = 512 // P
        NG = S // 512
        p_ex = tc.alloc_tile_pool(name="p_ex", bufs=4)
        p_sq = tc.alloc_tile_pool(name="p_sq", bufs=2)
        p_cen = tc.alloc_tile_pool(name="p_cen", bufs=4)
        p_rs = tc.alloc_tile_pool(name="p_rs", bufs=2)
        p_rsb = tc.alloc_tile_pool(name="p_rsb", bufs=2)
        p_wO = tc.alloc_tile_pool(name="p_wO", bufs=1)
        p_osb = tc.alloc_tile_pool(name="p_osb", bufs=4)
        p_scp = tc.alloc_tile_pool(name="p_scp", bufs=2, space="PSUM")
        p_ave = tc.alloc_tile_pool(name="p_ave", bufs=2, space="PSUM")
        p_eps = tc.alloc_tile_pool(name="p_eps", bufs=2, space="PSUM")
        p_psD = tc.alloc_tile_pool(name="p_psD", bufs=2, space="PSUM")

        wo_sb = []
        for h in range(HPC):
            w = p_wO.tile([P, OUT_COLS], BF16, name=f"wo_{h}", tag=f"wo_{h}")
            nc.scalar.dma_start(out=w[:], in_=wo_d[h * P:(h + 1) * P, :])
            wo_sb.append(w)

        for g in range(NG):
            cens = []
            for h in range(HPC):
                ave = p_ave.tile([P, 512], F32, name="ave", tag="ave", bufs=2)
                jmax = (g * GQ + GQ) if causal else NQT
                for j in range(jmax):
                    lo = max(0, j - g * GQ) if causal else 0
                    ncols = (GQ - lo) * P
                    c0 = g * 512 + lo * P
                    sc = p_scp.tile([P, 512], F32, name="sc", tag="sc", bufs=2)
                    nc.tensor.matmul(sc[:, :ncols], ksA[h][:, j * P:(j + 1) * P],
                                     qsA[h][:, c0:c0 + ncols], start=True, stop=False)
                    nc.tensor.matmul(sc[:, :ncols], ksB[h][:, j * P:(j + 1) * P],
                                     qsB[h][:, c0:c0 + ncols], start=False, stop=True)
                    ex = p_ex.tile([P, 512], F32R, name="ex", tag="ex", bufs=4)
                    nc.scalar.activation(ex[:, :ncols], sc[:, :ncols], AF.Exp,
                                         scale=exp_scale)
                    if causal and j >= g * GQ:
                        nc.vector.tensor_mul(ex[:, 0:P], ex[:, 0:P], diagmask[:])
                    nc.tensor.matmul(ave[:, lo * P:512], Vp[h][:, j, :],
                                     ex[:, :ncols], start=(j == 0),
                                     stop=(j == jmax - 1), skip_group_check=True)
                # epilogue: rs = 1/sqrt(|inner|), cenT = aveT * rs (free axis)
                sq = p_sq.tile([P, 512], F32, name="sq", tag="sq", bufs=2)
                nc.scalar.square(sq[:], ave[:])
                inn = p_eps.tile([P, 512], F32, name="inn", tag="eps", bufs=2)
                for t in range(GQ):
                    nc.tensor.matmul(inn[:, t:t + 1], sq[:, t * P:(t + 1) * P],
                                     sign_col_bf[:], start=(t == 0), stop=(t == 3),
                                     skip_group_check=True)
                # inner is strictly negative (future-timelike centroid), so
                # |inner| clamped = max(-inner, eps)
                xab = p_rs.tile([P, 4], F32, name="xab", tag="xab", bufs=2)
                nc.vector.tensor_scalar(xab[:], inn[:, 0:4], -1.0, EPS_DEN,
                                        op0=ALU.mult, op1=ALU.max)
                # fast rsqrt seed: bits(y) = MAGIC - bits(x)/2, done in float
                # (24-bit mantissa rounding of the bit-integers is ~1e-5 on the
                # seed, swallowed by the Newton steps)
                y = p_rs.tile([P, 4], F32, name="y", tag="y", bufs=2)
                t1 = p_rs.tile([P, 4], F32, name="t1", tag="t1", bufs=2)
                nc.vector.tensor_copy(t1[:], xab[:].bitcast(I32))
                nc.vector.tensor_scalar(t1[:], t1[:], -0.5, float(MAGIC),
                                        op0=ALU.mult, op1=ALU.add)
                nc.vector.tensor_copy(y[:].bitcast(I32), t1[:])
                for _ in range(2):
                    nc.vector.tensor_mul(t1[:], y[:], y[:])
                    nc.vector.tensor_mul(t1[:], xab[:], t1[:])
                    nc.vector.tensor_scalar(t1[:], t1[:], -0.5, 1.5,
                                            op0=ALU.mult, op1=ALU.add)
                    nc.vector.tensor_mul(y[:], y[:], t1[:])
                tpr = p_eps.tile([P, 512], F32, name="tpr", tag="eps", bufs=2)
                for t in range(GQ):
                    nc.tensor.transpose(tpr[0:1, t * P:(t + 1) * P],
                                        y[:, t:t + 1], identity[:])
                rs_row = p_rs.tile([1, 512], BF16, name="rs_row", tag="rs_row",
                                   bufs=2)
                nc.vector.tensor_copy(rs_row[:], tpr[0:1, 0:512])
                rsb = p_eps.tile([P, 512], F32, name="rsb", tag="eps", bufs=2)
                nc.tensor.matmul(rsb[:], ones_row_bf[:], rs_row[:],
                                 start=True, stop=True)
                rsb_sb = p_rsb.tile([P, 512], F32, name="rsb_sb", tag="rsb_sb", bufs=2)
                nc.scalar.copy(rsb_sb[:], rsb[:])
                cenT = p_cen.tile([P, 512], BF16, name=f"cenT_{h}", tag=f"cenT_{h}",
                                  bufs=2)
                nc.vector.tensor_mul(cenT[:], ave[:], rsb_sb[:])
                cens.append(cenT)
            # wo projection for this group's q-tiles (both heads done)
            for t in range(GQ):
                m = g * GQ + t
                ot = p_osb.tile([P, OUT_COLS], BF16, name="ot", tag="ot", bufs=3)
                for n in range(4):
                    n0 = n * 512
                    nn = min(512, OUT_COLS - n0)
                    psd = p_psD.tile([P, 512], F32, name="psd", tag="psd", bufs=2)
                    nc.tensor.matmul(psd[:, :nn], cens[0][:, t * P:(t + 1) * P],
                                     wo_sb[0][:, n0:n0 + nn], start=True, stop=False)
                    nc.tensor.matmul(psd[:, :nn], cens[1][:, t * P:(t + 1) * P],
                                     wo_sb[1][:, n0:n0 + nn], start=False, stop=True)
                    nc.any.tensor_copy(ot[:, n0:n0 + nn], psd[:, :nn])
                nc.gpsimd.dma_start(out=out_d[m * P:(m + 1) * P, :], in_=ot[:])

        p_psD.release()
        p_eps.release()
        p_ave.release()
        p_scp.release()
        p_osb.release()
        p_wO.release()
        p_rsb.release()
        p_rs.release()
        p_cen.release()
        p_sq.release()
        p_ex.release()

        big2.release()
        p_wA.release()
        big.release()
        const.release()

    nc.compile()
    return nc


_CACHE = {}


def _get_program(exp_scale: float, causal: bool):
    key = (round(float(exp_scale), 12), causal)
    if key not in _CACHE:
        _CACHE[key] = _build_program(float(exp_scale), causal)
    return _CACHE[key]


def _rope_perm():
    """Even rope dims first, then odd (host-side column permutation)."""
    return np.concatenate([np.arange(0, RSP, 2), np.arange(1, RSP, 2)])


def _prep_inputs(x, freqs_cos, freqs_sin, wq_w, wkv_a_w, kv_norm_w,
                 wkv_b_w, wo_w):
    x2 = np.ascontiguousarray(np.asarray(x, np.float32).reshape(S, DIM))
    xT = np.ascontiguousarray(x2.T)
    wq_w = np.asarray(wq_w, np.float32)
    wkv_a_w = np.asarray(wkv_a_w, np.float32)
    kv_norm_w = np.asarray(kv_norm_w, np.float32)
    wkv_b_w = np.asarray(wkv_b_w, np.float32)
    wo_w = np.asarray(wo_w, np.float32)
    cosT = np.asarray(freqs_cos, np.float32).T  # (32, S)
    sinT = np.asarray(freqs_sin, np.float32).T
    cosT = np.ascontiguousarray(np.concatenate([cosT, cosT], axis=0))
    sinT = np.ascontiguousarray(np.concatenate([sinT, -sinT], axis=0))

    rp = _rope_perm()
    # wq per core-pair layout: [nope_h0 | nope_h1 | rope_h0(ev,od) | rope_h1]
    wq_r = wq_w.reshape(DIM, 16, QH)
    wq_nope = wq_r[:, :, :NOPE]
    wq_rope = wq_r[:, :, NOPE:][:, :, rp]
    wq_cores = []
    for c in range(N_CORES):
        h0, h1 = 2 * c, 2 * c + 1
        wq_cores.append(np.concatenate(
            [wq_nope[:, h0], wq_nope[:, h1], wq_rope[:, h0], wq_rope[:, h1]],
            axis=1))
    # wkva: [kv | rope-even | rope-odd]
    wkva_p = wkv_a_w.copy()
    wkva_p[:, KV_RANK:] = wkva_p[:, KV_RANK:][:, rp]
    # wkvb: kvn rows first, time row last; cols [nope_h0|nope_h1|v_h0|v_h1]
    wkvb_p = np.ascontiguousarray(np.concatenate([wkv_b_w[1:], wkv_b_w[:1]], axis=0))
    wb_r = wkvb_p.reshape(KV_RANK + 1, 16, NOPE + VSP)
    wb_cores = []
    for c in range(N_CORES):
        h0, h1 = 2 * c, 2 * c + 1
        wb_cores.append(np.concatenate(
            [wb_r[:, h0, :NOPE], wb_r[:, h1, :NOPE],
             wb_r[:, h0, NOPE:], wb_r[:, h1, NOPE:]], axis=1))
    # wo rows per head: [v space (1..127), time (0)]
    wo_p = wo_w.reshape(16, P, OUT_COLS)
    wo_p = np.concatenate([wo_p[:, 1:, :], wo_p[:, 0:1, :]], axis=1)
    wo_p = wo_p.reshape(16 * P, OUT_COLS)

    xT_bf = np.ascontiguousarray(xT.astype(ml_dtypes.bfloat16))
    wkva_bf = np.ascontiguousarray(wkva_p.astype(ml_dtypes.bfloat16))
    wnormT = np.ascontiguousarray(kv_norm_w.reshape(4, P).T)
    cosT_bf = np.ascontiguousarray(cosT.astype(ml_dtypes.bfloat16))
    sinT_bf = np.ascontiguousarray(sinT.astype(ml_dtypes.bfloat16))

    signc = np.ones((P, 1), np.float32)
    signc[VSP, 0] = -1.0
    signc_bf = np.ascontiguousarray(signc)

    in_maps = []
    for c in range(N_CORES):
        in_maps.append({
            "signc": signc_bf,
            "xT": xT_bf,
            "wq": np.ascontiguousarray(wq_cores[c].astype(ml_dtypes.bfloat16)),
            "wkva": wkva_bf,
            "wnormT": wnormT,
            "wkvb": np.ascontiguousarray(wb_cores[c].astype(ml_dtypes.bfloat16)),
            "wo": np.ascontiguousarray(
                wo_p[c * WO_ROWS:(c + 1) * WO_ROWS, :].astype(ml_dtypes.bfloat16)),
            "cosT": cosT_bf,
            "sinT": sinT_bf,
            "xsl": np.ascontiguousarray(xT_bf[:, c * SL:(c + 1) * SL]),
            "cossl": np.ascontiguousarray(cosT_bf[:, c * SL:(c + 1) * SL]),
            "sinsl": np.ascontiguousarray(sinT_bf[:, c * SL:(c + 1) * SL]),
        })
    return in_maps


def kernel(x, start_pos, freqs_cos, freqs_sin, mask, wq_w, wkv_a_w, kv_norm_w,
           wkv_b_w, wo_w, softmax_scale, bias_p, _want_trace=False, _sim=False):
    mask = np.asarray(mask)
    causal = bool(np.array_equal(mask, np.triu(np.ones((S, S), bool), k=1)))
    if not causal:
        assert not mask.any(), "only causal or empty masks are supported"

    smax = float(np.asarray(softmax_scale).reshape(-1)[0])
    exp_scale = 2.0 / smax

    in_maps = _prep_inputs(x, freqs_cos, freqs_sin, wq_w, wkv_a_w, kv_norm_w,
                           wkv_b_w, wo_w)
    nc = _get_program(exp_scale, causal)

    if _sim:
        from concourse.bass_interp import MultiCoreSim
        sim = MultiCoreSim(nc, num_cores=N_CORES)
        for cid in range(N_CORES):
            cs = sim.cores[cid]
            for k, v in in_maps[cid].items():
                cs.tensor(k)[:] = v
        sim.simulate()
        outs = [np.asarray(sim.cores[c].tensor("out"), dtype=np.float32)
                for c in range(N_CORES)]
        kernel.last_result = None
    else:
        res = run_bass_kernel_spmd(nc, in_maps, core_ids=list(range(N_CORES)),
                                   trace=_want_trace)
        kernel.last_result = res
        outs = [res.results[c]["out"].astype(np.float32) for c in range(N_CORES)]

    total = outs[0]
    for c in range(1, N_CORES):
        total = total + outs[c]
    t = np.sqrt(np.sum(total * total, axis=-1, keepdims=True) + 1.0)
    out = np.concatenate([t, total], axis=-1)
    return out.reshape(1, S, DIM).astype(np.float32)
